# revision 10
# baseline (speedup 1.0000x reference)
import numpy as np
import ml_dtypes

import concourse.bass as bass
import concourse.tile as tile
from concourse import mybir
from concourse.bacc import Bacc
from concourse.bass_utils import run_bass_kernel_spmd

F32 = mybir.dt.float32
BF16 = mybir.dt.bfloat16
BF_NP = ml_dtypes.bfloat16

K = 5
TAPS = [(r, s) for r in range(5) for s in range(5) if not (r == 4 and s >= 2)]
NT = len(TAPS)  # 22

# w1x1 column offsets
KC1, KC2 = 0, 1280
QC1, QC2 = 3840, 4608
WQ, WK, WV = 6144, 6528, 7168
OC1, OCA, OC2 = 7808, 8320, 8576
W1X1_COLS = 9600

AL = mybir.AluOpType
AF = mybir.ActivationFunctionType


# ---------------- host-side weight prep ----------------

def _wn_conv(v, g, b, causal):
    v = np.asarray(v, np.float64).copy()
    if causal:
        v[:, :, -1, K // 2:] = 0.0
    w = np.asarray(g, np.float64).reshape(-1, 1, 1, 1) * v / np.sqrt(
        np.sum(v * v, axis=(1, 2, 3), keepdims=True))
    return w.astype(np.float32)


def _wn_lin(v, g):
    v = np.asarray(v, np.float64)
    w = np.asarray(g, np.float64)[:, None] * v / np.sqrt(
        np.sum(v * v, axis=1, keepdims=True))
    return w.astype(np.float32)


def _np_tree(p):
    if isinstance(p, dict):
        return {k: _np_tree(v) for k, v in p.items()}
    if isinstance(p, (list, tuple)):
        return [_np_tree(v) for v in p]
    return np.asarray(p, np.float32)


def _pack_conv_piece(w, co0):
    # w: [co_total, 256, 5, 5] weight-normed. returns [128, NT*2*128]
    arr = np.zeros((128, NT * 2 * 128), np.float32)
    for ti, (r, s) in enumerate(TAPS):
        for cit in range(2):
            blk = w[co0:co0 + 128, cit * 128:(cit + 1) * 128, r, s]  # [co, ci]
            arr[:, (ti * 2 + cit) * 128:(ti * 2 + cit + 1) * 128] = blk.T
    return arr


def _place(dst, col0, w, ci0, co0, nci=128, nco=128):
    # dst[ci, col0+co] = w[co0+co, ci0+ci]; w may be short in either dim
    blk = w[co0:co0 + nco, ci0:ci0 + nci]  # [<=nco, <=nci]
    dst[:blk.shape[1], col0:col0 + blk.shape[0]] = blk.T


def _prep(input, background, params):
    p = _np_tree(params)
    in_np = np.asarray(input, np.float32)
    bg_np = np.asarray(background, np.float32)

    # conv resblock weights
    wconv = np.zeros((24, 128, NT * 2 * 128), np.float32)
    for r in range(4):
        rp = p['res'][r]
        w1 = _wn_conv(rp['c1']['v'], rp['c1']['g'], rp['c1']['b'], True)
        w2 = _wn_conv(rp['c2']['v'], rp['c2']['g'], rp['c2']['b'], True)
        for t in range(2):
            wconv[r * 6 + t] = _pack_conv_piece(w1, t * 128)
        for t in range(4):
            wconv[r * 6 + 2 + t] = _pack_conv_piece(w2, t * 128)

    # 1x1 weights
    w1x1 = np.zeros((128, W1X1_COLS), np.float32)
    kb = p['key_rb']
    kc1 = _wn_conv(kb['c1']['v'], kb['c1']['g'], kb['c1']['b'], False)[:, :, 0, 0]
    kc2 = _wn_conv(kb['c2']['v'], kb['c2']['g'], kb['c2']['b'], False)[:, :, 0, 0]
    for ci in range(5):
        for co in range(2):
            _place(w1x1, KC1 + (ci * 2 + co) * 128, kc1, ci * 128, co * 128)
    for ci in range(2):
        for o in range(5):
            _place(w1x1, KC2 + (ci * 10 + o) * 128, kc2, ci * 128, o * 128)
            _place(w1x1, KC2 + (ci * 10 + 5 + o) * 128, kc2, ci * 128, 514 + o * 128)

    qb = p['query_rb']
    qc1 = _wn_conv(qb['c1']['v'], qb['c1']['g'], qb['c1']['b'], False)[:, :, 0, 0]
    qc2 = _wn_conv(qb['c2']['v'], qb['c2']['g'], qb['c2']['b'], False)[:, :, 0, 0]
    for ci in range(3):
        for co in range(2):
            _place(w1x1, QC1 + (ci * 2 + co) * 128, qc1, ci * 128, co * 128)
    for ci in range(2):
        for o in range(3):
            _place(w1x1, QC2 + (ci * 6 + o) * 128, qc2, ci * 128, o * 128)
            _place(w1x1, QC2 + (ci * 6 + 3 + o) * 128, qc2, ci * 128, 258 + o * 128)

    at = p['attn']
    wq = _wn_lin(at['q']['v'], at['q']['g'])            # [128,258]
    wk = _wn_lin(at['k']['v'], at['k']['g']) * 0.25     # [128,514]
    wv = _wn_lin(at['v']['v'], at['v']['g'])            # [128,514]
    for ci in range(3):
        _place(w1x1, WQ + ci * 128, wq, ci * 128, 0)
    for ci in range(5):
        _place(w1x1, WK + ci * 128, wk, ci * 128, 0)
        _place(w1x1, WV + ci * 128, wv, ci * 128, 0)

    ob = p['out_rb']
    oc1 = _wn_conv(ob['c1']['v'], ob['c1']['g'], ob['c1']['b'], False)[:, :, 0, 0]
    oca = _wn_conv(ob['ca']['v'], ob['ca']['g'], ob['ca']['b'], False)[:, :, 0, 0]
    oc2 = _wn_conv(ob['c2']['v'], ob['c2']['g'], ob['c2']['b'], False)[:, :, 0, 0]
    for ci in range(2):
        for co in range(2):
            _place(w1x1, OC1 + (ci * 2 + co) * 128, oc1, ci * 128, co * 128)
    for co in range(2):
        _place(w1x1, OCA + co * 128, oca, 0, co * 128)
    for ci in range(2):
        for o in range(4):
            _place(w1x1, OC2 + (ci * 4 + o) * 128, oc2, ci * 128, o * 128)

    # biases [128, 56]
    bias = np.zeros((128, 56), np.float32)

    def bcolv(c, vec):
        v = np.asarray(vec, np.float32)
        bias[:len(v), c] = v

    for r in range(4):
        b1 = p['res'][r]['c1']['b']
        b2 = p['res'][r]['c2']['b']
        for t in range(2):
            bcolv(6 * r + t, b1[t * 128:(t + 1) * 128])
        for o in range(2):
            bcolv(6 * r + 2 + o, b2[o * 128:(o + 1) * 128])
            bcolv(6 * r + 4 + o, b2[256 + o * 128:256 + (o + 1) * 128])
    bk1, bk2 = kb['c1']['b'], kb['c2']['b']
    for t in range(2):
        bcolv(24 + t, bk1[t * 128:(t + 1) * 128])
    for o in range(5):
        bcolv(26 + o, bk2[o * 128:min(514, (o + 1) * 128)])
        bcolv(31 + o, bk2[514 + o * 128:514 + min(514, (o + 1) * 128)])
    bq1, bq2 = qb['c1']['b'], qb['c2']['b']
    for t in range(2):
        bcolv(36 + t, bq1[t * 128:(t + 1) * 128])
    for o in range(3):
        bcolv(38 + o, bq2[o * 128:min(258, (o + 1) * 128)])
        bcolv(41 + o, bq2[258 + o * 128:258 + min(258, (o + 1) * 128)])
    bcolv(44, at['q']['b'])
    bcolv(45, np.asarray(at['k']['b'], np.float32) * 0.25)
    bcolv(46, at['v']['b'])
    bo1, boa, bo2 = ob['c1']['b'], ob['ca']['b'], ob['c2']['b']
    for t in range(2):
        bcolv(47 + t, bo1[t * 128:(t + 1) * 128] + boa[t * 128:(t + 1) * 128])
    for o in range(2):
        bcolv(49 + o, bo2[o * 128:(o + 1) * 128])
        bcolv(51 + o, bo2[256 + o * 128:256 + (o + 1) * 128])

    mask = np.fromfunction(lambda i, j: i < j, (128, 128)).astype(np.float32)
    ident = np.eye(128, dtype=np.float32)
    e8 = np.zeros((8, 128), np.float32)
    for j in range(8):
        e8[j, j * 16:(j + 1) * 16] = 1.0

    wconv_bf = wconv.astype(BF_NP)
    w1x1_bf = w1x1.astype(BF_NP)
    mask_bf = mask.astype(BF_NP)
    ident_bf = ident.astype(BF_NP)

    in_maps = []
    for c in range(8):
        in_maps.append({
            "x": np.ascontiguousarray(in_np[c]),
            "bg": np.ascontiguousarray(bg_np[c]),
            "wconv": wconv_bf,
            "w1x1": w1x1_bf,
            "bias": bias,
            "mask": mask_bf,
            "ident": ident_bf,
            "e8": e8,
        })
    return in_maps


# ---------------- device program ----------------

def build_nc():
    nc = Bacc()
    xd = nc.declare_dram_parameter("x", [256, 32, 32], F32, isOutput=False)
    bgd = nc.declare_dram_parameter("bg", [2, 32, 32], F32, isOutput=False)
    wconvd = nc.declare_dram_parameter("wconv", [24, 128, NT * 2 * 128], BF16,
                                       isOutput=False)
    w1x1d = nc.declare_dram_parameter("w1x1", [128, W1X1_COLS], BF16, isOutput=False)
    biasd = nc.declare_dram_parameter("bias", [128, 56], F32, isOutput=False)
    maskd = nc.declare_dram_parameter("mask", [128, 128], BF16, isOutput=False)
    identd = nc.declare_dram_parameter("ident", [128, 128], BF16, isOutput=False)
    e8d = nc.declare_dram_parameter("e8", [8, 128], F32, isOutput=False)
    outd = nc.declare_dram_parameter("out", [256, 32, 32], F32, isOutput=True)

    with tile.TileContext(nc) as tc:
        _emit(nc, tc, xd, bgd, wconvd, w1x1d, biasd, maskd, identd, e8d, outd)
    return nc


def _emit(nc, tc, xd, bgd, wconvd, w1x1d, biasd, maskd, identd, e8d, outd):
    with tc.tile_pool(name="persist", bufs=1) as P, \
         tc.tile_pool(name="scr", bufs=2) as SC, \
         tc.tile_pool(name="wconv_pool", bufs=2) as WC, \
         tc.tile_pool(name="kh_pool", bufs=2) as KH, \
         tc.tile_pool(name="e_pool", bufs=3) as EP, \
         tc.tile_pool(name="avs_pool", bufs=2) as AVS:

        # ---- persistent tiles ----
        ic = [P.tile([128, 32, 32], F32, tag=f"ic{t}", name=f"ic{t}") for t in range(2)]
        xa = [P.tile([128, 32, 32], F32, tag=f"xa{t}", name=f"xa{t}") for t in range(2)]
        xb = [P.tile([128, 32, 32], F32, tag=f"xb{t}", name=f"xb{t}") for t in range(2)]
        ex = [P.tile([128, 36, 36], BF16, tag=f"ex{t}", name=f"ex{t}") for t in range(2)]
        hp = [P.tile([128, 36, 36], BF16, tag=f"hp{t}", name=f"hp{t}") for t in range(2)]
        e4 = [P.tile([128, 32, 32], BF16, tag=f"e4{t}", name=f"e4{t}") for t in range(2)]
        ei = [P.tile([128, 32, 32], BF16, tag=f"ei{t}", name=f"ei{t}") for t in range(2)]
        bgz = P.tile([128, 32, 32], F32, tag="bgz", name="bgz")
        ebg = P.tile([128, 32, 32], BF16, tag="ebg", name="ebg")
        keyt = [P.tile([128, 32, 32], BF16, tag=f"key{t}", name=f"key{t}")
                for t in range(5)]
        qryt = [P.tile([128, 32, 32], BF16, tag=f"qry{t}", name=f"qry{t}")
                for t in range(3)]
        h1 = [P.tile([128, 32, 32], BF16, tag=f"h1_{t}", name=f"h1_{t}")
              for t in range(2)]
        q_sb = P.tile([128, 1024], BF16, tag="q_sb", name="q_sb")
        k_sb = P.tile([128, 1024], BF16, tag="k_sb", name="k_sb")
        v_sb = P.tile([128, 1024], BF16, tag="v_sb", name="v_sb")
        v17 = [P.tile([128, 8, 17], BF16, tag=f"v17_{t}", name=f"v17_{t}")
               for t in range(8)]
        attn_un = P.tile([128, 1024], F32, tag="attn_un", name="attn_un")
        sums = P.tile([8, 1024], F32, tag="sums", name="sums")
        rec = P.tile([8, 1024], F32, tag="rec", name="rec")
        attn_f = P.tile([128, 32, 32], BF16, tag="attn_f", name="attn_f")
        bias_sb = P.tile([128, 56], F32, tag="bias_sb", name="bias_sb")
        mask_sb = P.tile([128, 128], BF16, tag="mask_sb", name="mask_sb")
        ident_sb = P.tile([128, 128], BF16, tag="ident_sb", name="ident_sb")
        e8_sb = P.tile([8, 128], F32, tag="e8_sb", name="e8_sb")
        w1x1_sb = P.tile([128, W1X1_COLS], BF16, tag="w1x1_sb", name="w1x1_sb")

        def bcol(c):
            return bias_sb[:, c:c + 1]

        # ---- initial DMA ----
        nc.gpsimd.dma_start(bias_sb[:], biasd[:])
        nc.gpsimd.dma_start(mask_sb[:], maskd[:])
        nc.gpsimd.dma_start(ident_sb[:], identd[:])
        nc.gpsimd.dma_start(e8_sb[:], e8d[:])
        for t in range(2):
            nc.gpsimd.dma_start(ic[t][:], xd[t * 128:(t + 1) * 128])
        nc.vector.memset(bgz[:], 0.0)
        nc.gpsimd.dma_start(bgz[0:2, :, :], bgd[:])
        nc.sync.dma_start(w1x1_sb[:], w1x1d[:])

        for t in range(2):
            nc.vector.memset(ex[t][:], 0.0)
            nc.vector.memset(hp[t][:], 0.0)

        # ---- helpers ----
        def elu_full(dst_ap, src_ap):
            # dst = elu(src); [128,1024]-sized APs
            rt = SC.tile([128, 32, 32], F32, tag="frt", name="frt")
            mt = SC.tile([128, 32, 32], F32, tag="fmt", name="fmt")
            et = SC.tile([128, 32, 32], F32, tag="fet", name="fet")
            nc.scalar.activation(rt[:], src_ap, AF.Relu)
            nc.vector.tensor_scalar(mt[:], src_ap, 0.0, 0.0, AL.add, AL.min)
            nc.scalar.activation(et[:], mt[:], AF.Exp)
            nc.vector.scalar_tensor_tensor(dst_ap, et[:], -1.0, rt[:], AL.add, AL.add)

        def elu_chunk(dst_ap, src_ap, bias_col):
            # dst = elu(src + bias); [128,512]-sized APs
            rt = SC.tile([128, 16, 32], F32, tag="crt", name="crt")
            mt = SC.tile([128, 16, 32], F32, tag="cmt", name="cmt")
            et = SC.tile([128, 16, 32], F32, tag="cet", name="cet")
            nc.scalar.activation(rt[:], src_ap, AF.Relu, bias=bias_col)
            nc.vector.tensor_scalar(mt[:], src_ap, bias_col, 0.0, AL.add, AL.min)
            nc.scalar.activation(et[:], mt[:], AF.Exp)
            nc.vector.scalar_tensor_tensor(dst_ap, et[:], -1.0, rt[:], AL.add, AL.add)

        def glu_chunk(dst_ap, psA, psB, ba, bb, res_ap):
            # dst = (psA+ba)*sigmoid(psB+bb) + res; [128,512]-sized APs
            gt = SC.tile([128, 16, 32], F32, tag="gt", name="gt")
            tt = SC.tile([128, 16, 32], F32, tag="tt", name="tt")
            nc.scalar.activation(gt[:], psB, AF.Sigmoid, bias=bb)
            nc.vector.scalar_tensor_tensor(tt[:], psA, ba, gt[:], AL.add, AL.mult)
            nc.vector.tensor_tensor(dst_ap, tt[:], res_ap, AL.add)

        # elu of raw input / bg (used by key_rb)
        elu_full(ebg[:], bgz[:])
        for t in range(2):
            elu_full(ei[t][:], ic[t][:])

        # ---- 4 causal conv resblocks ----
        def conv_accum(ps, wc, src):
            for ti, (r, s) in enumerate(TAPS):
                for cit in range(2):
                    first = (ti == 0 and cit == 0)
                    last = (ti == NT - 1 and cit == 1)
                    lhsT = wc[:, (ti * 2 + cit) * 128:(ti * 2 + cit + 1) * 128]
                    for ch in range(2):
                        rhs = src[cit][:, 16 * ch + r:16 * ch + r + 16, s:s + 32]
                        nc.tensor.matmul(ps[ch][:], lhsT, rhs, start=first, stop=last)

        with tc.tile_pool(name="conv_psum", bufs=6, space="PSUM") as CPS:
            x_cur = ic
            x_seq = [xa, xb, xa, xb]
            for r in range(4):
                x_nxt = x_seq[r]
                for t in range(2):
                    elu_full(ex[t][:, 4:36, 2:34], x_cur[t][:])
                for co in range(2):
                    wc = WC.tile([128, NT * 2 * 128], BF16, tag="wc", name="wc")
                    nc.sync.dma_start(wc[:], wconvd[r * 6 + co])
                    ps = [CPS.tile([128, 16, 32], F32, tag="cps", name="cps")
                          for _ in range(2)]
                    conv_accum(ps, wc, ex)
                    for ch in range(2):
                        elu_chunk(hp[co][:, 4 + 16 * ch:20 + 16 * ch, 2:34],
                                  ps[ch][:], bcol(6 * r + co))
                for o in range(2):
                    wca = WC.tile([128, NT * 2 * 128], BF16, tag="wc", name="wc")
                    nc.sync.dma_start(wca[:], wconvd[r * 6 + 2 + o])
                    psA = [CPS.tile([128, 16, 32], F32, tag="cps", name="cps")
                           for _ in range(2)]
                    conv_accum(psA, wca, hp)
                    wcb = WC.tile([128, NT * 2 * 128], BF16, tag="wc", name="wc")
                    nc.sync.dma_start(wcb[:], wconvd[r * 6 + 4 + o])
                    psB = [CPS.tile([128, 16, 32], F32, tag="cps", name="cps")
                           for _ in range(2)]
                    conv_accum(psB, wcb, hp)
                    for ch in range(2):
                        glu_chunk(x_nxt[o][:, 16 * ch:16 * ch + 16, :],
                                  psA[ch][:], psB[ch][:],
                                  bcol(6 * r + 2 + o), bcol(6 * r + 4 + o),
                                  x_cur[o][:, 16 * ch:16 * ch + 16, :])
                x_cur = x_nxt

        x4 = x_cur  # == xb
        for t in range(2):
            elu_full(e4[t][:], x4[t][:])

        # ---- 1x1 resblocks + attention ----
        with tc.tile_pool(name="pp_psum", bufs=3, space="PSUM") as PPS:

            def mm1x1(ps_ap, ci_tiles, col0, stride, co, ch):
                n = len(ci_tiles)
                for i, t in enumerate(ci_tiles):
                    c0 = col0 + (i * stride + co) * 128
                    nc.tensor.matmul(ps_ap, w1x1_sb[:, c0:c0 + 128],
                                     t[:, 16 * ch:16 * ch + 16, :],
                                     start=(i == 0), stop=(i == n - 1))

            def rb1x1(eK, resK, c1_col, c1_bias0, c2_col, c2_stride, na,
                      a_bias0, b_bias0, out_tiles, h_tiles):
                for co in range(2):
                    for ch in range(2):
                        ps = PPS.tile([128, 16, 32], F32, tag="pps", name="pps")
                        mm1x1(ps[:], eK, c1_col, 2, co, ch)
                        elu_chunk(h_tiles[co][:, 16 * ch:16 * ch + 16, :],
                                  ps[:], bcol(c1_bias0 + co))
                for o in range(na):
                    for ch in range(2):
                        psA = PPS.tile([128, 16, 32], F32, tag="pps", name="pps")
                        mm1x1(psA[:], h_tiles, c2_col, c2_stride, o, ch)
                        psB = PPS.tile([128, 16, 32], F32, tag="pps", name="pps")
                        mm1x1(psB[:], h_tiles, c2_col, c2_stride, na + o, ch)
                        glu_chunk(out_tiles[o][:, 16 * ch:16 * ch + 16, :],
                                  psA[:], psB[:], bcol(a_bias0 + o), bcol(b_bias0 + o),
                                  resK[o][:, 16 * ch:16 * ch + 16, :])

            eK = [ei[0], ei[1], e4[0], e4[1], ebg]
            resK = [ic[0], ic[1], x4[0], x4[1], bgz]
            rb1x1(eK, resK, KC1, 24, KC2, 10, 5, 26, 31, keyt, h1)
            eQ = [e4[0], e4[1], ebg]
            resQ = [x4[0], x4[1], bgz]
            rb1x1(eQ, resQ, QC1, 36, QC2, 6, 3, 38, 41, qryt, h1)

            # attention projections -> q/k/v [128ch, 1024px] bf16
            def proj(dst, ci_tiles, col0, bias_c):
                for ch in range(2):
                    ps = PPS.tile([128, 16, 32], F32, tag="pps", name="pps")
                    n = len(ci_tiles)
                    for i, t in enumerate(ci_tiles):
                        c0 = col0 + i * 128
                        nc.tensor.matmul(ps[:], w1x1_sb[:, c0:c0 + 128],
                                         t[:, 16 * ch:16 * ch + 16, :],
                                         start=(i == 0), stop=(i == n - 1))
                    nc.vector.tensor_scalar(dst[:, 512 * ch:512 * (ch + 1)], ps[:],
                                            bias_c, None, AL.add)

            proj(q_sb, qryt, WQ, bcol(44))
            proj(k_sb, keyt, WK, bcol(45))
            proj(v_sb, keyt, WV, bcol(46))

            # V17: per key-tile transpose of v, +ones row for softmax sums
            with tc.tile_pool(name="v_psum", bufs=2, space="PSUM") as VPS:
                for kt in range(8):
                    pt = VPS.tile([128, 128], BF16, tag="vt", name="vt")
                    nc.tensor.transpose(pt[:], v_sb[:, 128 * kt:128 * (kt + 1)],
                                        ident_sb[:])
                    nc.vector.tensor_copy(v17[kt][:, :, 0:16], pt[:])
                    nc.vector.memset(v17[kt][:, :, 16:17], 1.0)

            # attention heads
            with tc.tile_pool(name="a_psum", bufs=3, space="PSUM") as APS, \
                 tc.tile_pool(name="av_psum", bufs=1, space="PSUM") as AVPS:
                for h in range(8):
                    k_h = KH.tile([16, 1024], BF16, tag="k_h", name="k_h")
                    q_h = KH.tile([16, 1024], BF16, tag="q_h", name="q_h")
                    nc.gpsimd.dma_start(k_h[:], k_sb[16 * h:16 * h + 16, :])
                    nc.gpsimd.dma_start(q_h[:], q_sb[16 * h:16 * h + 16, :])
                    avA = AVPS.tile([17, 512], F32, tag="avA", name="avA")
                    avB = AVPS.tile([17, 512], F32, tag="avB", name="avB")
                    for kt in range(8):
                        base = 128 * kt
                        et = EP.tile([128, 1024], BF16, tag="et", name="et")
                        chunks = ([(base, 512), (512, 1024)] if base < 512
                                  else [(base, 1024)])
                        for (c0, c1) in chunks:
                            sp = APS.tile([128, 512], F32, tag="sp", name="sp")
                            nc.tensor.matmul(sp[:, 0:c1 - c0],
                                             k_h[:, base:base + 128],
                                             q_h[:, c0:c1], start=True, stop=True)
                            nc.scalar.activation(et[:, c0:c1], sp[:, 0:c1 - c0],
                                                 AF.Exp)
                        nc.vector.tensor_tensor(et[:, base:base + 128],
                                                et[:, base:base + 128],
                                                mask_sb[:], AL.mult)
                        lhsT = v17[kt][:, h:h + 1, :]
                        if base < 512:
                            a0 = base if kt > 0 else 0
                            nc.tensor.matmul(avA[:, a0:512], lhsT, et[:, a0:512],
                                             start=(kt == 0), stop=(kt == 3),
                                             skip_group_check=True)
                            nc.tensor.matmul(avB[:, 0:512], lhsT, et[:, 512:1024],
                                             start=(kt == 0), stop=False,
                                             skip_group_check=True)
                        else:
                            b0 = base - 512 if kt > 4 else 0
                            nc.tensor.matmul(avB[:, b0:512], lhsT,
                                             et[:, 512 + b0:1024],
                                             start=False, stop=(kt == 7),
                                             skip_group_check=True)
                    av_sb = AVS.tile([17, 1024], F32, tag="av_sb", name="av_sb")
                    nc.vector.tensor_copy(av_sb[:, 0:512], avA[:])
                    nc.vector.tensor_copy(av_sb[:, 512:1024], avB[:])
                    nc.gpsimd.dma_start(attn_un[16 * h:16 * h + 16, :], av_sb[0:16, :])
                    nc.gpsimd.dma_start(sums[h:h + 1, :], av_sb[16:17, :])

            # softmax normalize + elu
            nc.vector.tensor_scalar(rec[:], sums[:], 1e-30, None, AL.max)
            nc.vector.reciprocal(rec[:], rec[:])
            with tc.tile_pool(name="r_psum", bufs=2, space="PSUM") as RPS:
                for ch in range(2):
                    rb = RPS.tile([128, 512], F32, tag="rb", name="rb")
                    nc.tensor.matmul(rb[:], e8_sb[:], rec[:, 512 * ch:512 * (ch + 1)],
                                     start=True, stop=True)
                    nc.vector.tensor_tensor(attn_un[:, 512 * ch:512 * (ch + 1)],
                                            attn_un[:, 512 * ch:512 * (ch + 1)],
                                            rb[:], AL.mult)
            elu_full(attn_f[:], attn_un[:])

            # out_rb
            eO = [e4[0], e4[1], attn_f]
            for co in range(2):
                for ch in range(2):
                    ps = PPS.tile([128, 16, 32], F32, tag="pps", name="pps")
                    for i, t in enumerate(eO):
                        if i < 2:
                            c0 = OC1 + (i * 2 + co) * 128
                        else:
                            c0 = OCA + co * 128
                        nc.tensor.matmul(ps[:], w1x1_sb[:, c0:c0 + 128],
                                         t[:, 16 * ch:16 * ch + 16, :],
                                         start=(i == 0), stop=(i == 2))
                    elu_chunk(h1[co][:, 16 * ch:16 * ch + 16, :], ps[:], bcol(47 + co))
            out_sb = [xa[0], xa[1]]  # reuse
            for o in range(2):
                for ch in range(2):
                    psA = PPS.tile([128, 16, 32], F32, tag="pps", name="pps")
                    mm1x1(psA[:], h1, OC2, 4, o, ch)
                    psB = PPS.tile([128, 16, 32], F32, tag="pps", name="pps")
                    mm1x1(psB[:], h1, OC2, 4, o + 2, ch)
                    glu_chunk(out_sb[o][:, 16 * ch:16 * ch + 16, :],
                              psA[:], psB[:], bcol(49 + o), bcol(51 + o),
                              x4[o][:, 16 * ch:16 * ch + 16, :])
        for o in range(2):
            nc.sync.dma_start(outd[o * 128:(o + 1) * 128], out_sb[o][:])


# ---------------- entry point ----------------

def kernel(**inputs):
    in_maps = _prep(inputs["input"], inputs["background"], inputs["params"])
    nc = build_nc()
    nc.finalize()
    res = run_bass_kernel_spmd(nc, in_maps, core_ids=list(range(8)))
    out = np.stack([np.asarray(r["out"], np.float32) for r in res.results])
    return out
